# revision 1
# baseline (speedup 1.0000x reference)
"""GCN layer on 8 trn2 NeuronCores.

out = segment_sum((h @ W * norm)[src], dst) * norm + bias

Key algebra: (h@W)*norm = (h*norm)@W and segment_sum is linear, so
out = (segment_sum(h[src] * norm[src] * norm[dst], dst) @ W) + bias.
Both norm factors ride on the EDGE weight ew = norm[src]*norm[dst], applied
inside a one-hot selection matrix; the scatter-sum is PSUM-accumulated
matmuls and the 128x128 weight GEMM runs once per output tile.

v3 over the f32 baseline: bf16 h table and bf16 one-hot/matmul pipeline
(4x PE rate, half the gather bytes), both norm factors folded into the edge
weight on the host, PSUM evictions + bias add moved to the scalar (ACT)
engine with the final GEMM emitted transposed ([feat, node], host
untransposes), dstl/ew interleaved in one f32 tensor, local-search tile
assignment to shrink 128-padding of per-(slot,quad) gather segments.
Gathers stay per-(slot,quad) with single_packet=True: big multi-slot
gathers need single_packet=False (one giant packet overflows the
descriptor ring and deadlocks the DMA engine -> NRT timeout).

Sharding: nodes padded to 784 tiles of 128; edges partitioned by dst tile.
Tiles are rank-sorted by edge count and dealt across the 8 cores so that
each program slot has near-equal padded edge counts on every core (one
shared SPMD program). h is split into 4 quadrant views of 25088 rows so
dma_gather's int16 indices can address them.
"""
import numpy as np
import ml_dtypes

import concourse.bass as bass
import concourse.mybir as mybir
import concourse.tile as tile
from concourse import bacc
from concourse.bass_utils import run_bass_kernel_spmd
from concourse.library_config import mlp

P = 128
N = 100000
E = 1600000
NCORES = 8
NT = 784                # node tiles after padding (784*128 = 100352)
NPAD = NT * P
TPC = NT // NCORES      # tiles (slots) per core = 98
Q = 4                   # quadrant tables for int16 gather indices
R = NPAD // Q           # 25088 rows per quadrant
G = 7                   # slots per gather group
NG = TPC // G           # 14 groups
GATHER_GROUPED = False  # True: one gather per (group, quad) w/ single_packet
                        # off; False: per (slot, quad) gathers (v1-proven)

_cache = {}
LOOPS = 1            # device-side repetitions of the kernel body (timing instrument)
RUN_KWARGS = {}      # test.py may set {"trace": True} etc.
LAST_RESULTS = None  # BassKernelResults of the last run
EMULATE = False      # numpy emulation of the device program (debug)
LAST_NC = None
LAST_IN_MAPS = None

BF = mybir.dt.bfloat16


def _groups():
    return [[g + NG * i for i in range(G)] for g in range(NG)]


def _build_program(K_sq, Csq):
    """Build the shared SPMD Bass program.

    K_sq[s][q]: padded (x128) gather count for slot s, quadrant q.
    Csq[s][q]:  K_sq//128 chunks.
    """
    C_s = [sum(Csq[s]) for s in range(TPC)]     # chunks per slot
    idx_cols = sum(sum(K_sq[s]) for s in range(TPC)) // 16
    chunk_cols = sum(C_s)

    nc = bacc.Bacc(None, target_bir_lowering=False)
    f32 = mybir.dt.float32
    hq_d = [nc.dram_tensor(f"h{q}", [R, P], BF, kind="ExternalInput")
            for q in range(Q)]
    idx_d = nc.dram_tensor("idx16", [P, idx_cols], mybir.dt.int16,
                           kind="ExternalInput")
    dw_d = nc.dram_tensor("dw", [P, 2 * chunk_cols], f32, kind="ExternalInput")
    bb_d = nc.dram_tensor("bb", [P, 1], f32, kind="ExternalInput")
    w_d = nc.dram_tensor("wt", [P, P], BF, kind="ExternalInput")
    out_d = nc.dram_tensor("out", [TPC * P, P], f32, kind="ExternalOutput")
    out_v = out_d.rearrange("(t p) d -> t p d", p=P)

    with tile.TileContext(nc) as tc:
        with (
            tc.tile_pool(name="const", bufs=1) as cpool,
            tc.tile_pool(name="gather", bufs=2) as gpool,
            tc.tile_pool(name="pt", bufs=8) as ptpool,
            tc.tile_pool(name="ps", bufs=4, space="PSUM") as pspool,
            tc.tile_pool(name="ps2", bufs=2, space="PSUM") as ps2pool,
            tc.tile_pool(name="oo", bufs=3) as opool,
            tc.tile_pool(name="agg", bufs=3) as aggpool,
        ):
            nc.gpsimd.load_library(mlp)
            idx_sb = cpool.tile([P, idx_cols], mybir.dt.int16)
            nc.sync.dma_start(idx_sb[:], idx_d[:])
            dw_sb = cpool.tile([P, 2 * chunk_cols], f32)
            nc.sync.dma_start(dw_sb[:], dw_d[:])
            bb_sb = cpool.tile([P, 1], f32)
            nc.sync.dma_start(bb_sb[:], bb_d[:])
            w_sb = cpool.tile([P, P], BF)
            nc.sync.dma_start(w_sb[:], w_d[:])
            iota_i = cpool.tile([P, P], mybir.dt.int32)
            nc.gpsimd.iota(iota_i[:], pattern=[[1, P]], base=0,
                           channel_multiplier=0)
            iota_f = cpool.tile([P, P], mybir.dt.float32)
            nc.vector.tensor_copy(iota_f[:], iota_i[:])

            for _rep in range(LOOPS):
                icol = 0   # idx col offset, (g, q) call order
                ccol = 0   # dw chunk counter, (g, s, q, c) order
                for slots in _groups():
                    C_g = sum(C_s[s] for s in slots)
                    msgs = gpool.tile([P, C_g, P], BF, tag="msgs")
                    # chunk base of (q, s) inside msgs: quad-major, slot-minor
                    base = {}
                    off = 0
                    for q in range(Q):
                        for s in slots:
                            base[(q, s)] = off
                            off += Csq[s][q]
                    for q in range(Q):
                        Kg = sum(K_sq[s][q] for s in slots)
                        if Kg == 0:
                            continue
                        if GATHER_GROUPED:
                            b0 = base[(q, slots[0])]
                            nc.gpsimd.dma_gather(
                                msgs[:, b0:b0 + Kg // P, :], hq_d[q][:],
                                idx_sb[:, icol:icol + Kg // 16], Kg, Kg, P,
                                single_packet=False,
                            )
                            icol += Kg // 16
                        else:
                            for s in slots:
                                K = K_sq[s][q]
                                if K == 0:
                                    continue
                                b0 = base[(q, s)]
                                nc.gpsimd.dma_gather(
                                    msgs[:, b0:b0 + K // P, :], hq_d[q][:],
                                    idx_sb[:, icol:icol + K // 16], K, K, P,
                                )
                                icol += K // 16
                    for s in slots:
                        aggT_ps = pspool.tile([P, P], mybir.dt.float32, tag="agg")
                        nch = C_s[s]
                        done = 0
                        for q in range(Q):
                            for c in range(Csq[s][q]):
                                pt = ptpool.tile([P, P], BF, tag="pt")
                                nc.vector.tensor_scalar(
                                    pt[:], iota_f[:],
                                    dw_sb[:, 2 * ccol:2 * ccol + 1],
                                    dw_sb[:, 2 * ccol + 1:2 * ccol + 2],
                                    op0=mybir.AluOpType.is_equal,
                                    op1=mybir.AluOpType.mult,
                                )
                                nc.tensor.matmul(
                                    aggT_ps[:], lhsT=msgs[:, base[(q, s)] + c, :],
                                    rhs=pt[:],
                                    start=(done == 0), stop=(done == nch - 1),
                                )
                                ccol += 1
                                done += 1
                        aggT_sb = aggpool.tile([P, P], BF, tag="aggT")
                        nc.scalar.copy(aggT_sb[:], aggT_ps[:])
                        # outT = W.T @ agg = [feat_out, node]; bias rides the
                        # partition dim so ACT applies it during PSUM eviction
                        outT_ps = ps2pool.tile([P, P], mybir.dt.float32, tag="out")
                        nc.tensor.matmul(outT_ps[:], lhsT=w_sb[:], rhs=aggT_sb[:],
                                         start=True, stop=True)
                        o_sb = opool.tile([P, P], mybir.dt.float32, tag="o")
                        nc.scalar.activation(
                            o_sb[:], outT_ps[:],
                            mybir.ActivationFunctionType.Identity,
                            bias=bb_sb[:, 0:1], scale=1.0)
                        nc.sync.dma_start(out_v[s], o_sb[:])
    nc.compile()
    return nc


def _optimize_assignment(counts, tiles_sc, iters=120000, seed=0):
    """Local-search swaps minimizing total padded gather count.

    Cost per slot = sum_q ceil(max_core cnt/128)*128; tiles can sit in any
    (slot, core) cell since the host maps outputs back via tiles_sc.
    """
    rng = np.random.default_rng(seed)
    tsc = tiles_sc.copy()
    cnt = counts[tsc].astype(np.int64)             # [s, c, q]

    def slot_cost(a):
        return int(((a.max(0) + P - 1) // P * P).sum())

    costs = np.array([slot_cost(cnt[s]) for s in range(TPC)])
    s1s = rng.integers(0, TPC, iters)
    s2s = rng.integers(0, TPC, iters)
    c1s = rng.integers(0, NCORES, iters)
    c2s = rng.integers(0, NCORES, iters)
    for s1, s2, c1, c2 in zip(s1s, s2s, c1s, c2s):
        if s1 == s2:
            continue
        a1 = cnt[s1]
        a2 = cnt[s2]
        r1 = a1[c1].copy()
        r2 = a2[c2].copy()
        a1[c1] = r2
        a2[c2] = r1
        n1 = slot_cost(a1)
        n2 = slot_cost(a2)
        if n1 + n2 < costs[s1] + costs[s2]:
            costs[s1] = n1
            costs[s2] = n2
            t = tsc[s1, c1]
            tsc[s1, c1] = tsc[s2, c2]
            tsc[s2, c2] = t
        else:
            a1[c1] = r1
            a2[c2] = r2
    return tsc


def kernel(h, norm, src, dst, weight, bias):
    h = np.ascontiguousarray(h, dtype=np.float32)
    norm = np.ascontiguousarray(norm, dtype=np.float32).reshape(-1)
    src = np.ascontiguousarray(src, dtype=np.int64).reshape(-1)
    dst = np.ascontiguousarray(dst, dtype=np.int64).reshape(-1)
    weight = np.ascontiguousarray(weight, dtype=np.float32)
    bias = np.ascontiguousarray(bias, dtype=np.float32).reshape(-1)
    n, d = h.shape
    e = src.shape[0]
    assert (n, d, e) == (N, P, E), (n, d, e)

    h_pad = np.zeros((NPAD, P), np.float32)
    h_pad[:n] = h
    h_bf = h_pad.astype(ml_dtypes.bfloat16)
    hq = [np.ascontiguousarray(h_bf[q * R:(q + 1) * R]) for q in range(Q)]

    tile_id = dst // P
    dstl_all = (dst % P).astype(np.float32)
    quad = src // R
    srcl_all = (src % R).astype(np.int16)
    ew_all = (norm[src] * norm[dst]).astype(np.float32)

    key = tile_id * Q + quad
    order = np.argsort(key, kind="stable")
    counts = np.bincount(key, minlength=NT * Q).reshape(NT, Q)
    starts = np.zeros((NT, Q), np.int64)
    starts.reshape(-1)[1:] = np.cumsum(counts.reshape(-1))[:-1]

    # rank-matched slot assignment: sort tiles by total count (desc), deal
    # rank r to core r%8, slot r//8 -> the 8 tiles at a slot have similar
    # counts, minimizing per-slot max padding.
    totals = counts.sum(1)
    rank = np.argsort(-totals, kind="stable")
    tiles_sc = rank.reshape(TPC, NCORES)           # [slot][core] -> tile id
    tiles_sc = _optimize_assignment(counts, tiles_sc)

    cnt_sc = counts[tiles_sc]                      # [slot][core][quad]
    K_sq = ((cnt_sc.max(axis=1) + P - 1) // P * P).astype(np.int64)  # [s][q]
    Csq = (K_sq // P).astype(np.int64)
    C_s = Csq.sum(1)
    idx_cols = int(K_sq.sum()) // 16
    chunk_cols = int(C_s.sum())

    if not EMULATE:
        key_prog = (tuple(map(tuple, K_sq)), LOOPS, GATHER_GROUPED)
        if key_prog not in _cache:
            _cache[key_prog] = _build_program(K_sq.tolist(), Csq.tolist())
        nc = _cache[key_prog]

    srcl_ord = srcl_all[order]
    dstl_ord = dstl_all[order]
    ew_ord = ew_all[order]

    groups = _groups()
    in_maps = []
    for c in range(NCORES):
        idx16 = np.zeros((16, idx_cols), np.int16)  # replicated x8 below
        dw = np.zeros((P, 2 * chunk_cols), np.float32)
        icol = 0
        ccol = 0
        for slots in groups:
            for q in range(Q):
                Kg = int(sum(K_sq[s, q] for s in slots))
                if Kg == 0:
                    continue
                seg = np.zeros((Kg,), np.int16)
                off = 0
                for s in slots:
                    K = int(K_sq[s, q])
                    t = tiles_sc[s, c]
                    cnt = int(counts[t, q])
                    st = int(starts[t, q])
                    seg[off:off + cnt] = srcl_ord[st:st + cnt]
                    off += K
                idx16[:, icol:icol + Kg // 16] = seg.reshape(Kg // 16, 16).T
                icol += Kg // 16
            for s in slots:
                for q in range(Q):
                    K = int(K_sq[s, q])
                    if K == 0:
                        continue
                    cq = int(Csq[s, q])
                    t = tiles_sc[s, c]
                    cnt = int(counts[t, q])
                    st = int(starts[t, q])
                    seg_d = np.zeros((cq * P,), np.float32)
                    seg_d[:cnt] = dstl_ord[st:st + cnt]
                    seg_w = np.zeros((cq * P,), np.float32)
                    seg_w[:cnt] = ew_ord[st:st + cnt]
                    dw[:, 2 * ccol:2 * (ccol + cq):2] = seg_d.reshape(cq, P).T
                    dw[:, 2 * ccol + 1:2 * (ccol + cq) + 1:2] = \
                        seg_w.reshape(cq, P).T
                    ccol += cq
        in_maps.append({
            "h0": hq[0], "h1": hq[1], "h2": hq[2], "h3": hq[3],
            "idx16": np.tile(idx16, (8, 1)),
            "dw": dw,
            "bb": bias.reshape(P, 1).astype(np.float32),
            "wt": weight.astype(ml_dtypes.bfloat16),
        })

    global LAST_NC, LAST_IN_MAPS
    LAST_NC, LAST_IN_MAPS = (nc if not EMULATE else None), in_maps
    if EMULATE:
        results = [_emulate_core(m, K_sq, Csq) for m in in_maps]
    else:
        res = run_bass_kernel_spmd(nc, in_maps, core_ids=list(range(NCORES)),
                                   **RUN_KWARGS)
        global LAST_RESULTS
        LAST_RESULTS = res
        results = [res.results[c]["out"] for c in range(NCORES)]

    out_tiles = np.zeros((NT, P, P), np.float32)
    for c in range(NCORES):
        out_tiles[tiles_sc[:, c]] = \
            results[c].reshape(TPC, P, P).transpose(0, 2, 1)
    return out_tiles.reshape(NPAD, P)[:N].copy()


def _emulate_core(m, K_sq, Csq):
    """Numpy emulation of the device program (mirrors _build_program)."""
    hq = np.stack([m[f"h{q}"].astype(np.float32) for q in range(Q)])
    iota = np.arange(P, dtype=np.float32)[None, :]          # [1,128]
    dw = m["dw"].astype(np.float32)
    wt = m["wt"].astype(np.float32)
    C_s = Csq.sum(1)
    out = np.zeros((TPC, P, P), np.float32)
    icol = 0
    ccol = 0
    for slots in _groups():
        C_g = int(sum(C_s[s] for s in slots))
        msgs = np.zeros((C_g, P, P), np.float32)
        base = {}
        off = 0
        for q in range(Q):
            for s in slots:
                base[(q, s)] = off
                off += int(Csq[s, q])
        for q in range(Q):
            Kg = int(sum(K_sq[s, q] for s in slots))
            if Kg == 0:
                continue
            idx = m["idx16"][:16, icol:icol + Kg // 16].T.reshape(-1)
            g = hq[q][idx]                                  # [Kg,128]
            b0 = base[(q, slots[0])]
            msgs[b0:b0 + Kg // P] = g.reshape(Kg // P, P, P)
            icol += Kg // 16
        for s in slots:
            aggT = np.zeros((P, P), np.float32)
            for q in range(Q):
                for c in range(int(Csq[s, q])):
                    dstl = dw[:, 2 * ccol][:, None]         # [128,1]
                    ew = dw[:, 2 * ccol + 1][:, None]
                    pt = (iota == dstl).astype(np.float32) * ew
                    aggT += msgs[base[(q, s)] + c].T @ pt   # [feat, node]
                    ccol += 1
            o = (wt.T @ aggT) + m["bb"]          # [feat_out, node]
            out[s] = o
    return out

